# revision 1
# baseline (speedup 1.0000x reference)
"""Trainium2 Bass kernel for nn_ApproxExp_FXP32in16out14 (histogram_binning).

Reference semantics: fixed-point piecewise-linear LUT approximation of exp(x)
over 17 uniform breakpoints on [-10, 4] (FXP32.16 in, FXP16.14 out), including
int32-wraparound artifacts of the torch reference in segments 14/15.

The LUT values y0[k] = rint(2^14 exp(-10+0.875k)) are geometric to ~0.35% for
the segments that contain data, and the interpolation weight is affine in x, so
the whole map factors as

    out(x) ~= exp(0.875*k - c0) * ((8/7)*x - k + c1),   k = rne((8/7)*x + 153/14)

which runs as 2 ScalarE activation passes (int32-RNE quantize; table via Exp)
and 2 DVE scalar_tensor_tensor passes, fully overlapped with the DMA streams
(memory-bound). A deterministic ~0.3% of elements (the int32-wraparound bands
at x>=2.7773, the x>=4 clamp, deep tail x<-4.7) is recomputed exactly on host.

Sharding: pure data parallel, leading dim 64 -> 8 cores x 8.
"""

import math
import os
from contextlib import ExitStack

import numpy as np

import concourse.bass as bass
import concourse.mybir as mybir
from concourse.bass_utils import run_bass_kernel_spmd

# ---------------------------------------------------------------- constants
FULL_SHAPE = (64, 4096, 1024)
N_CORES = 8
TILES, P, F = 64, 128, 4096  # per-core: 64 tiles of [128, 4096] fp32

RHO = math.exp(0.875) - 1.0
CONST = 1.0 + RHO / 32768.0          # +0.5 LSB rounding offset of t_fx in Q14
B_SL = RHO / CONST                   # k-coefficient before unit-rescale
CONST1 = 1.0 + (655360.0 / 57344.0) * RHO / CONST
AK_SCALE = 8.0 / 7.0                 # 65536/57344
AK_BIAS = 153.0 / 14.0               # 655360/57344 - 0.5
A2_SCALE = 0.875
A2_BIAS = -10.0 + math.log(CONST) + math.log(B_SL)
T3_ADD = CONST1 / B_SL               # (V0 + T3_ADD) * y2S'

# host-fixup region boundaries (float32 compares on raw x)
FIX_HI = np.float32(2.7773)          # below first int32-wrap threshold (2.77735)
FIX_LO = np.float32(-4.7)            # deep tail: LUT quantization breaks the model

# ------------------------------------------------------------ bass builder
_NC = None


def _build_nc() -> bass.Bass:
    global _NC
    if _NC is not None:
        return _NC
    f32, i32 = mybir.dt.float32, mybir.dt.int32
    nc = bass.Bass()
    x_ext = nc.declare_dram_parameter("x", [TILES, P, F], f32, isOutput=False)
    o_ext = nc.declare_dram_parameter("out", [TILES, P, F], f32, isOutput=True)

    # [128,1] constant for the Exp activation bias (const_aps only has 0/1).
    bias_t = nc.alloc_sbuf_tensor("const-a2bias", [P, 1], f32)
    nc.gpsimd.memset(bias_t.ap(), A2_BIAS)
    nc.all_engine_barrier()
    a2_bias_ap = bias_t.ap()

    ctx = ExitStack()
    xt = [ctx.enter_context(nc.sbuf_tensor(f"xt{j}", [P, F], f32)) for j in range(2)]
    kq = [ctx.enter_context(nc.sbuf_tensor(f"kq{j}", [P, F], i32)) for j in range(2)]
    ys = [ctx.enter_context(nc.sbuf_tensor(f"ys{j}", [P, F], f32)) for j in range(2)]
    vt = [ctx.enter_context(nc.sbuf_tensor(f"vt{j}", [P, F], f32)) for j in range(2)]
    s_in = ctx.enter_context(nc.semaphore("s_in"))
    s_k = ctx.enter_context(nc.semaphore("s_k"))
    s_y = ctx.enter_context(nc.semaphore("s_y"))
    s_v1 = ctx.enter_context(nc.semaphore("s_v1"))
    s_o = ctx.enter_context(nc.semaphore("s_o"))
    s_out = ctx.enter_context(nc.semaphore("s_out"))
    block = ctx.enter_context(nc.Block())

    @block.sync
    def _(sync):
        for i in range(TILES):
            if i >= 2:
                sync.wait_ge(s_out, 16 * (i - 1))
            sync.dma_start(out=xt[i % 2][:], in_=x_ext[i]).then_inc(s_in, 16)
            if i >= 1:
                sync.wait_ge(s_o, i)
                sync.dma_start(out=o_ext[i - 1], in_=xt[(i - 1) % 2][:]).then_inc(s_out, 16)
        sync.wait_ge(s_o, TILES)
        sync.dma_start(out=o_ext[TILES - 1], in_=xt[(TILES - 1) % 2][:]).then_inc(s_out, 16)

    @block.scalar
    def _(scalar):
        for i in range(TILES):
            scalar.wait_ge(s_in, 16 * (i + 1))
            if i >= 2:
                scalar.wait_ge(s_v1, i - 1)  # kq slot free (T2(i-2) done)
            nc.scalar.activation(
                kq[i % 2][:], xt[i % 2][:], mybir.ActivationFunctionType.Copy,
                bias=AK_BIAS, scale=AK_SCALE,
            ).then_inc(s_k, 1)
            if i >= 2:
                scalar.wait_ge(s_o, i - 1)  # ys slot free (T3(i-2) done)
            nc.scalar.activation(
                ys[i % 2][:], kq[i % 2][:], mybir.ActivationFunctionType.Exp,
                bias=a2_bias_ap, scale=A2_SCALE,
            ).then_inc(s_y, 1)

    @block.vector
    def _(vector):
        for i in range(TILES):
            vector.wait_ge(s_in, 16 * (i + 1))
            vector.wait_ge(s_k, i + 1)
            # T2: V0 = x*(8/7) - kq
            nc.vector.scalar_tensor_tensor(
                out=vt[i % 2][:], in0=xt[i % 2][:], scalar=AK_SCALE, in1=kq[i % 2][:],
                op0=mybir.AluOpType.mult, op1=mybir.AluOpType.subtract,
            ).then_inc(s_v1, 1)
            vector.wait_ge(s_y, i + 1)
            # T3: out = (V0 + T3_ADD) * y2S'
            nc.vector.scalar_tensor_tensor(
                out=xt[i % 2][:], in0=vt[i % 2][:], scalar=T3_ADD, in1=ys[i % 2][:],
                op0=mybir.AluOpType.add, op1=mybir.AluOpType.mult,
            ).then_inc(s_o, 1)

    ctx.close()
    _NC = nc
    return nc


# ------------------------------------------------- exact host-side reference
_XP = np.round(np.linspace(-10.0, 4.0, 17) * 65536.0).astype(np.int64)
_YV = np.round(np.exp(np.linspace(-10.0, 4.0, 17)) * 16384.0).astype(np.int64)
_DY = np.diff(_YV)


def _reference_exact(xs: np.ndarray) -> np.ndarray:
    """Bit-faithful int32 reference for a (small) subset of elements."""
    x_int = np.rint(xs.astype(np.float64) * 65536.0).astype(np.int64)
    mask_low = x_int <= _XP[0]
    mask_high = x_int >= _XP[-1]
    xc = np.clip(x_int, _XP[0], _XP[-1])
    idx = np.clip(np.searchsorted(_XP, xc, side="left") - 1, 0, 15)
    dxv = xc - _XP[idx]
    t_fx = ((dxv << 14) + 28672) // 57344
    prod = t_fx * _DY[idx] + 8192
    pm = prod & 0xFFFFFFFF
    S = np.where(pm >= 1 << 31, pm - (1 << 32), pm)
    interp = _YV[idx] + (S >> 14)
    out_int = np.where(mask_low, _YV[0], np.where(mask_high, _YV[-1], interp))
    return (out_int.astype(np.float32) / np.float32(16384.0)).astype(np.float32)


def _host_fixup(x_flat: np.ndarray, out_flat: np.ndarray) -> None:
    sel = (x_flat >= FIX_HI) | (x_flat < FIX_LO)
    idxs = np.flatnonzero(sel)
    if idxs.size:
        out_flat[idxs] = _reference_exact(x_flat[idxs])


_last_results = None


def kernel(x: np.ndarray) -> np.ndarray:
    assert x.shape == FULL_SHAPE and x.dtype == np.float32, (x.shape, x.dtype)
    nc = _build_nc()
    per = FULL_SHAPE[0] // N_CORES
    in_maps = [
        {"x": np.ascontiguousarray(x[i * per : (i + 1) * per]).reshape(TILES, P, F)}
        for i in range(N_CORES)
    ]
    global _last_results
    res = run_bass_kernel_spmd(nc, in_maps, core_ids=list(range(N_CORES)))
    _last_results = res
    out = np.concatenate(
        [r["out"].reshape(per, FULL_SHAPE[1], FULL_SHAPE[2]) for r in res.results],
        axis=0,
    )
    _host_fixup(x.ravel(), out.ravel())
    return out



# revision 6
# speedup vs baseline: 1.2760x; 1.2760x over previous
"""Trainium2 Bass kernel for nn_ApproxExp_FXP32in16out14 (histogram_binning).

Reference semantics: fixed-point piecewise-linear LUT approximation of exp(x)
over 17 uniform breakpoints on [-10, 4] (FXP32.16 in, FXP16.14 out), including
int32-wraparound artifacts of the torch reference in segments 14/15.

The LUT values y0[k] = rint(2^14 exp(-10+0.875k)) are geometric to ~0.35% for
the segments that contain data, and the interpolation weight is affine in x, so
the whole map factors as

    out(x) ~= exp(0.875*k - c0) * ((8/7)*x - k + c1),   k = rne((8/7)*x + 153/14)

which runs as 2 ScalarE activation passes (int32-RNE quantize; table via Exp)
and 2 DVE scalar_tensor_tensor passes, fully overlapped with the DMA streams
(memory-bound). The result is stored as bf16 (halves the output HBM traffic;
bf16 rounding is ~0.1% rel RMS, far under the 2e-2 gate) and upcast on host.
A deterministic ~0.3% of elements (the int32-wraparound bands at x>=2.7773,
the x>=4 clamp, deep tail x<-4.7) is recomputed exactly on host.

Sharding: pure data parallel, leading dim 64 -> 8 cores x 8.
"""

import math
import os
from contextlib import ExitStack

import numpy as np

import concourse.bass as bass
import concourse.mybir as mybir
from concourse.bass_utils import run_bass_kernel_spmd

# ---------------------------------------------------------------- constants
FULL_SHAPE = (64, 4096, 1024)
N_CORES = 8
TILES, P, F = 64, 128, 4096  # per-core: 64 tiles of [128, 4096] fp32
NBUF = 3

RHO = math.exp(0.875) - 1.0
CONST = 1.0 + RHO / 32768.0          # +0.5 LSB rounding offset of t_fx in Q14
B_SL = RHO / CONST                   # k-coefficient before unit-rescale
CONST1 = 1.0 + (655360.0 / 57344.0) * RHO / CONST
AK_SCALE = 8.0 / 7.0                 # 65536/57344
AK_BIAS = 153.0 / 14.0               # 655360/57344 - 0.5
A2_SCALE = 0.875
A2_BIAS = -10.0 + math.log(CONST) + math.log(B_SL)
T3_ADD = CONST1 / B_SL               # (V0 + T3_ADD) * y2S'

# host-fixup region boundaries (float32 compares on raw x)
FIX_HI = np.float32(2.7773)          # below first int32-wrap threshold (2.77735)
FIX_LO = np.float32(-4.7)            # deep tail: LUT quantization breaks the model

# ------------------------------------------------------------ bass builder
_NC = None


def _build_nc(tiles: int = TILES) -> bass.Bass:
    f32, i32, bf16 = mybir.dt.float32, mybir.dt.int32, mybir.dt.bfloat16
    nc = bass.Bass()
    x_ext = nc.declare_dram_parameter("x", [tiles, P, F], f32, isOutput=False)
    o_ext = nc.declare_dram_parameter("out", [tiles, P, F], bf16, isOutput=True)

    # [128,1] constant for the Exp activation bias (const_aps only has 0/1).
    bias_t = nc.alloc_sbuf_tensor("const-a2bias", [P, 1], f32)
    nc.gpsimd.memset(bias_t.ap(), A2_BIAS)
    nc.all_engine_barrier()
    a2_bias_ap = bias_t.ap()

    ctx = ExitStack()
    xt = [ctx.enter_context(nc.sbuf_tensor(f"xt{j}", [P, F], f32)) for j in range(NBUF)]
    kq = [ctx.enter_context(nc.sbuf_tensor(f"kq{j}", [P, F], i32)) for j in range(NBUF)]
    ys = [ctx.enter_context(nc.sbuf_tensor(f"ys{j}", [P, F], f32)) for j in range(NBUF)]
    ot = [ctx.enter_context(nc.sbuf_tensor(f"ot{j}", [P, F], bf16)) for j in range(NBUF)]
    # per-buffer-slot DMA semaphores: at most one in-flight DMA per sem, so a
    # waiter on >=16*n can't be satisfied by interleaved partial completions
    # of two DMAs (the 16 per-engine increments of concurrent DMAs interleave).
    s_in = [ctx.enter_context(nc.semaphore(f"s_in{j}")) for j in range(NBUF)]
    s_out = [ctx.enter_context(nc.semaphore(f"s_out{j}")) for j in range(NBUF)]
    s_k = ctx.enter_context(nc.semaphore("s_k"))
    s_y = ctx.enter_context(nc.semaphore("s_y"))
    s_v = ctx.enter_context(nc.semaphore("s_v"))
    s_o = ctx.enter_context(nc.semaphore("s_o"))
    block = ctx.enter_context(nc.Block())

    @block.sync
    def _(sync):
        for i in range(min(2, tiles)):
            sync.dma_start(out=xt[i % NBUF][:], in_=x_ext[i]).then_inc(
                s_in[i % NBUF], 16
            )
        for i in range(tiles):
            if i + 2 < tiles:
                # xt[(i+2)%NBUF] holds V0 of tile i-1 until T3(i-1) reads it
                if i >= 1:
                    sync.wait_ge(s_o, i)
                sync.dma_start(
                    out=xt[(i + 2) % NBUF][:], in_=x_ext[i + 2]
                ).then_inc(s_in[(i + 2) % NBUF], 16)
            sync.wait_ge(s_o, i + 1)
            sync.dma_start(out=o_ext[i], in_=ot[i % NBUF][:]).then_inc(
                s_out[i % NBUF], 16
            )

    @block.scalar
    def _(scalar):
        for i in range(tiles):
            j = i % NBUF
            scalar.wait_ge(s_in[j], 16 * (i // NBUF + 1))
            if i >= NBUF:
                scalar.wait_ge(s_v, i - NBUF + 1)  # kq slot free (T2(i-3) done)
            nc.scalar.activation(
                kq[j][:], xt[j][:], mybir.ActivationFunctionType.Copy,
                bias=AK_BIAS, scale=AK_SCALE,
            ).then_inc(s_k, 1)
            if i >= NBUF:
                scalar.wait_ge(s_o, i - NBUF + 1)  # ys slot free (T3(i-3) done)
            scalar.wait_ge(s_k, i + 1)  # own K(i) retired (race-detector sync)
            nc.scalar.activation(
                ys[j][:], kq[j][:], mybir.ActivationFunctionType.Exp,
                bias=a2_bias_ap, scale=A2_SCALE,
            ).then_inc(s_y, 1)

    @block.vector
    def _(vector):
        for i in range(tiles):
            j = i % NBUF
            vector.wait_ge(s_k, i + 1)
            # T2 (in-place): xt = x*(8/7) - kq
            nc.vector.scalar_tensor_tensor(
                out=xt[j][:], in0=xt[j][:], scalar=AK_SCALE,
                in1=kq[j][:],
                op0=mybir.AluOpType.mult, op1=mybir.AluOpType.subtract,
            ).then_inc(s_v, 1)
            vector.wait_ge(s_y, i + 1)
            vector.wait_ge(s_v, i + 1)  # own T2(i) retired (race-detector sync)
            if i >= NBUF:
                vector.wait_ge(s_out[j], 16 * (i // NBUF))  # ot slot free
            # T3: out = (V0 + T3_ADD) * y2S'   (bf16 output)
            nc.vector.scalar_tensor_tensor(
                out=ot[j][:], in0=xt[j][:], scalar=T3_ADD,
                in1=ys[j][:],
                op0=mybir.AluOpType.add, op1=mybir.AluOpType.mult,
            ).then_inc(s_o, 1)

    ctx.close()
    return nc


def _get_nc() -> bass.Bass:
    global _NC
    if _NC is None:
        _NC = _build_nc()
    return _NC


# ------------------------------------------------- exact host-side reference
_XP = np.round(np.linspace(-10.0, 4.0, 17) * 65536.0).astype(np.int64)
_YV = np.round(np.exp(np.linspace(-10.0, 4.0, 17)) * 16384.0).astype(np.int64)
_DY = np.diff(_YV)


def _reference_exact(xs: np.ndarray) -> np.ndarray:
    """Bit-faithful int32 reference for a (small) subset of elements."""
    x_int = np.rint(xs.astype(np.float64) * 65536.0).astype(np.int64)
    mask_low = x_int <= _XP[0]
    mask_high = x_int >= _XP[-1]
    xc = np.clip(x_int, _XP[0], _XP[-1])
    idx = np.clip(np.searchsorted(_XP, xc, side="left") - 1, 0, 15)
    dxv = xc - _XP[idx]
    t_fx = ((dxv << 14) + 28672) // 57344
    prod = t_fx * _DY[idx] + 8192
    pm = prod & 0xFFFFFFFF
    S = np.where(pm >= 1 << 31, pm - (1 << 32), pm)
    interp = _YV[idx] + (S >> 14)
    out_int = np.where(mask_low, _YV[0], np.where(mask_high, _YV[-1], interp))
    return (out_int.astype(np.float32) / np.float32(16384.0)).astype(np.float32)


def _host_fixup(x_flat: np.ndarray, out_flat: np.ndarray) -> None:
    sel = (x_flat >= FIX_HI) | (x_flat < FIX_LO)
    idxs = np.flatnonzero(sel)
    if idxs.size:
        out_flat[idxs] = _reference_exact(x_flat[idxs])


_last_results = None


def kernel(x: np.ndarray) -> np.ndarray:
    assert x.shape == FULL_SHAPE and x.dtype == np.float32, (x.shape, x.dtype)
    nc = _get_nc()
    per = FULL_SHAPE[0] // N_CORES
    in_maps = [
        {"x": np.ascontiguousarray(x[i * per : (i + 1) * per]).reshape(TILES, P, F)}
        for i in range(N_CORES)
    ]
    global _last_results
    res = run_bass_kernel_spmd(nc, in_maps, core_ids=list(range(N_CORES)))
    _last_results = res
    out = np.concatenate(
        [
            r["out"].astype(np.float32).reshape(per, FULL_SHAPE[1], FULL_SHAPE[2])
            for r in res.results
        ],
        axis=0,
    )
    _host_fixup(x.ravel(), out.ravel())
    return out
